# revision 11
# baseline (speedup 1.0000x reference)
"""AesSA Trainium kernel v2: 2 cores per sample, query-axis sharded attention.

Key differences from v1:
- k/vT (phase 1) and gk/hvT/hv2T (phase 2) are SBUF-resident (no DRAM round trips).
- Logits in [n, m] layout (queries in partitions): rowmax/exp/rowsum are
  per-partition ops (DVE reduce_max negate, Act Exp with bias + accum_out).
- Flash-style per-(n-chunk, m-chunk) local max; normalization and the
  exp(local-max) rescale folded into the transpose step as a diagonal-matrix
  matmul (LT = Texp^T @ diag(f_j/rowsum)).
- Value-path matmuls (transpose, AV, mean, second) in fp16.
- Content stats computed from this core's half + pairwise AllGather.
"""
import sys
sys.path.insert(0, '/opt/trn_rl_repo')
import numpy as np
import concourse.bacc as bacc
import concourse.mybir as mybir
import concourse.tile as tile
from contextlib import ExitStack

dt = mybir.dt
AF = mybir.ActivationFunctionType
AX = mybir.AxisListType
ALU = mybir.AluOpType

C = 512
CT = 4
EPS = 1e-5
LDT = dt.float16    # logits-path matmul operand dtype
VDT = dt.float16    # value-path matmul operand dtype

NS = 4096
NH = NS // 2
MT = NS // 128      # 32 m-tiles
MW = 512            # m-chunk width (f32r needs >=256 free for full rate)
MCH = NS // MW      # 8 m-chunks
NCH = 256           # n-chunk width
NQ = NCH // 128     # 2 n-tiles per n-chunk
QC = NH // NCH      # 8 n-chunks per core


def build(n_cores=8, pairs=None):
    if pairs is None:
        pairs = [[2 * i, 2 * i + 1] for i in range(n_cores // 2)]

    nc = bacc.Bacc("TRN2", target_bir_lowering=False, debug=False, num_devices=n_cores)

    styleD = nc.dram_tensor("style", [C, NS], LDT, kind="ExternalInput")
    style_hD = nc.dram_tensor("style_h", [C, NH], LDT, kind="ExternalInput")
    aesD = nc.dram_tensor("aes", [C, NS], LDT, kind="ExternalInput")
    cont_hD = nc.dram_tensor("cont_h", [C, NH], dt.float32, kind="ExternalInput")
    WT = {}
    for w in ["f1", "f2", "f3", "frs", "f", "g", "h"]:
        WT[w] = nc.dram_tensor(f"WT{w}", [C, C], LDT, kind="ExternalInput")
    BCOL = {}
    for w in ["f1", "f2", "frs", "f", "g"]:
        BCOL[w] = nc.dram_tensor(f"bcol{w}", [128, CT], dt.float32, kind="ExternalInput")
    BROW = {}
    for w in ["f3", "h"]:
        BROW[w] = nc.dram_tensor(f"brow{w}", [1, C], dt.float32, kind="ExternalInput")
    identD = nc.dram_tensor("ident", [128, 128], dt.float32, kind="ExternalInput")
    outD = nc.dram_tensor("out", [C, NH], dt.float32, kind="ExternalOutput")

    with tile.TileContext(nc, num_cores=n_cores) as tc, ExitStack() as octx:
        dram = octx.enter_context(tc.tile_pool(name="dram", bufs=1, space="DRAM"))
        sk_halfD = dram.tile([QC, C, NCH], LDT, name="sk_halfD")
        sk_fullD = dram.tile([QC, 2, C, NCH], LDT, name="sk_fullD")
        statPD = dram.tile([128, 2 * CT], dt.float32, name="statPD")
        statFD = dram.tile([2, 128, 2 * CT], dt.float32, name="statFD")

        cpool = octx.enter_context(tc.tile_pool(name="const", bufs=1))
        psum = octx.enter_context(tc.tile_pool(name="psum", bufs=1, space="PSUM"))

        # ---- constants ----
        ident = cpool.tile([128, 128], dt.float32, name="ident")
        nc.sync.dma_start(ident[:], identD[:])
        ident_h = cpool.tile([128, 128], VDT, name="ident_h")
        nc.vector.tensor_copy(ident_h[:], ident[:])
        bcol = {}
        for w in BCOL:
            bcol[w] = cpool.tile([128, CT], dt.float32, name=f"bcol{w}")
            nc.sync.dma_start(bcol[w][:], BCOL[w][:])
        brow = {}
        for w in BROW:
            r = cpool.tile([1, C], dt.float32, name=f"brow{w}_r")
            nc.sync.dma_start(r[:], BROW[w][:])
            brow[w] = cpool.tile([128, C], dt.float32, name=f"brow{w}")
            nc.gpsimd.partition_broadcast(brow[w][:], r[:])
        zero_b = cpool.tile([128, 1], dt.float32, name="zero_b")
        nc.gpsimd.memset(zero_b[:], 0.0)
        eps_b = cpool.tile([128, 1], dt.float32, name="eps_b")
        nc.gpsimd.memset(eps_b[:], EPS)
        mean_t = cpool.tile([128, CT], dt.float32, name="mean_t")
        rstd_t = cpool.tile([128, CT], dt.float32, name="rstd_t")

        # =========================== PHASE 1 ===========================
        with ExitStack() as ph1:
            wpool = ph1.enter_context(tc.tile_pool(name="w_ph1", bufs=1))
            res1 = ph1.enter_context(tc.tile_pool(name="res1", bufs=1))
            Wf1 = [wpool.tile([128, C], LDT, name=f"Wf1_{i}") for i in range(CT)]
            Wfrs = [wpool.tile([128, C], LDT, name=f"Wfrs_{i}") for i in range(CT)]
            for i in range(CT):
                nc.sync.dma_start(Wf1[i][:], WT["f1"][i * 128:(i + 1) * 128, :])
                nc.sync.dma_start(Wfrs[i][:], WT["frs"][i * 128:(i + 1) * 128, :])
            kres = [[res1.tile([128, MW], LDT, name=f"kres{i}_{j}") for j in range(MCH)]
                    for i in range(CT)]
            vres = res1.tile([128, MT, C], VDT, name="vres")

            # ---- phase 1a: k + vT build, content-half stats ----
            with ExitStack() as ph1a:
                wb = ph1a.enter_context(tc.tile_pool(name="w_build", bufs=1))
                rota = ph1a.enter_context(tc.tile_pool(name="rot1a", bufs=2))
                Wf2 = [wb.tile([128, C], LDT, name=f"Wf2_{i}") for i in range(CT)]
                Wf3 = [wb.tile([128, C], LDT, name=f"Wf3_{i}") for i in range(CT)]
                for i in range(CT):
                    nc.sync.dma_start(Wf2[i][:], WT["f2"][i * 128:(i + 1) * 128, :])
                    nc.sync.dma_start(Wf3[i][:], WT["f3"][i * 128:(i + 1) * 128, :])
                for mch in range(MCH):
                    arot = [rota.tile([128, MW], LDT, name=f"arot{it}") for it in range(CT)]
                    for it in range(CT):
                        nc.gpsimd.dma_start(
                            arot[it][:], aesD[it * 128:(it + 1) * 128, mch * MW:(mch + 1) * MW])
                    for ot in range(CT):
                        ps = psum.tile([128, MW], dt.float32, name="ps_k", tag="big4", bufs=3)
                        for it in range(CT):
                            nc.tensor.matmul(ps[:], Wf2[it][:, ot * 128:(ot + 1) * 128],
                                             arot[it][:], start=(it == 0), stop=(it == CT - 1))
                        if ot % 2 == 0:
                            nc.scalar.activation(kres[ot][mch][:], ps[:],
                                                 AF.Identity, bias=bcol["f2"][:, ot:ot + 1])
                        else:
                            nc.vector.tensor_scalar_add(kres[ot][mch][:], ps[:],
                                                        bcol["f2"][:, ot:ot + 1])
                    for ms in range(MW // 128):
                        mt = mch * (MW // 128) + ms
                        psv = psum.tile([128, C], dt.float32, name="ps_v", tag="big4", bufs=3)
                        for it in range(CT):
                            nc.tensor.matmul(psv[:], arot[it][:, ms * 128:(ms + 1) * 128],
                                             Wf3[it][:], start=(it == 0), stop=(it == CT - 1))
                        nc.vector.tensor_add(vres[:, mt, :], psv[:], brow["f3"][:])
                # content-half stats + pairwise exchange
                NSC = NH // MW  # 4 chunks of 512 for stats
                sumst = rota.tile([128, CT, NSC], dt.float32, name="sumst", bufs=1)
                sqst = rota.tile([128, CT, NSC], dt.float32, name="sqst", bufs=1)
                for sc in range(NSC):
                    for ct in range(CT):
                        crot = rota.tile([128, MW], dt.float32, name="crot")
                        nc.gpsimd.dma_start(
                            crot[:], cont_hD[ct * 128:(ct + 1) * 128, sc * MW:(sc + 1) * MW])
                        nc.vector.reduce_sum(sumst[:, ct, sc:sc + 1], crot[:], axis=AX.X)
                        sq_scr = rota.tile([128, MW], dt.float32, name="sq_scr")
                        nc.scalar.activation(sq_scr[:], crot[:], AF.Square, bias=zero_b[:],
                                             accum_out=sqst[:, ct, sc:sc + 1])
                statP = rota.tile([128, 2 * CT], dt.float32, name="statP", bufs=1)
                for ct in range(CT):
                    nc.vector.reduce_sum(statP[:, ct:ct + 1], sumst[:, ct, :], axis=AX.X)
                    nc.vector.reduce_sum(statP[:, CT + ct:CT + ct + 1], sqst[:, ct, :], axis=AX.X)
                nc.sync.dma_start(statPD[:], statP[:])
                nc.gpsimd.collective_compute(
                    "AllGather", ALU.bypass, replica_groups=pairs,
                    ins=[statPD[:].opt()], outs=[statFD[:].opt()])
                statF = rota.tile([128, 2, 2 * CT], dt.float32, name="statF", bufs=1)
                for hh in range(2):
                    nc.sync.dma_start(statF[:, hh, :], statFD[hh])
                tot = rota.tile([128, 2 * CT], dt.float32, name="tot", bufs=1)
                nc.vector.tensor_add(tot[:], statF[:, 0, :], statF[:, 1, :])
                nc.vector.tensor_scalar_mul(mean_t[:], tot[:, 0:CT], 1.0 / NS)
                ex2 = rota.tile([128, CT], dt.float32, name="ex2", bufs=1)
                nc.vector.tensor_scalar_mul(ex2[:], tot[:, CT:2 * CT], 1.0 / NS)
                msq = rota.tile([128, CT], dt.float32, name="msq_t", bufs=1)
                nc.vector.tensor_mul(msq[:], mean_t[:], mean_t[:])
                var_t = rota.tile([128, CT], dt.float32, name="var_t", bufs=1)
                nc.vector.tensor_sub(var_t[:], ex2[:], msq[:])
                sd_t = rota.tile([128, CT], dt.float32, name="sd_t", bufs=1)
                nc.scalar.activation(sd_t[:], var_t[:], AF.Sqrt, bias=eps_b[:])
                nc.vector.reciprocal(rstd_t[:], sd_t[:])

            # ---- phase 1b: attention 1, [n, m] layout ----
            with ExitStack() as ph1b:
                rot = ph1b.enter_context(tc.tile_pool(name="rot1b", bufs=2))
                spool = ph1b.enter_context(tc.tile_pool(name="small1", bufs=2))

                def head1(qc):
                    sh = rot.tile([128, CT, NCH], LDT, name="sh", bufs=2)
                    for it in range(CT):
                        nc.gpsimd.dma_start(
                            sh[:, it, :], style_hD[it * 128:(it + 1) * 128, qc * NCH:(qc + 1) * NCH])
                    q = rot.tile([128, CT, NCH], LDT, name="q", bufs=2)
                    for ot in range(CT):
                        psq = psum.tile([128, NCH], dt.float32, name="ps_q", tag="big4", bufs=3)
                        for it in range(CT):
                            nc.tensor.matmul(psq[:], Wf1[it][:, ot * 128:(ot + 1) * 128],
                                             sh[:, it, :], start=(it == 0), stop=(it == CT - 1))
                        nc.vector.tensor_scalar_add(q[:, ot, :], psq[:], bcol["f1"][:, ot:ot + 1])
                    sm = _sm_head(nc, psum, rot, spool, q,
                                  lambda ct, j: kres[ct][j][:], ident_h, name="1")
                    return sh, sm

                def tail1(qc, sh, sm):
                    LT = _sm_tailT(nc, psum, rot, sm, name="1")
                    # AV: xn[c, n] = sum_m vT[m, c] * LT[m, n]  (pre-normalized)
                    xn = rot.tile([128, CT, NCH], LDT, name="xn", bufs=1)
                    for ct in range(CT):
                        ps_av = psum.tile([128, NCH], dt.float32, name="ps_av", tag="big4", bufs=3)
                        for mt in range(MT):
                            nc.tensor.matmul(ps_av[:], vres[:, mt, ct * 128:(ct + 1) * 128],
                                             LT[:, mt, :], start=(mt == 0), stop=(mt == MT - 1))
                        nc.vector.tensor_copy(xn[:, ct, :], ps_av[:])
                    # Wfrs conv + residual -> sk chunk -> collective
                    for ot in range(CT):
                        ps = psum.tile([128, NCH], dt.float32, name="ps_sk", tag="big4", bufs=3)
                        for it in range(CT):
                            nc.tensor.matmul(ps[:], Wfrs[it][:, ot * 128:(ot + 1) * 128],
                                             xn[:, it, :], start=(it == 0), stop=(it == CT - 1))
                        sk0 = rot.tile([128, NCH], dt.float32, name="sk0")
                        nc.vector.tensor_scalar_add(sk0[:], ps[:], bcol["frs"][:, ot:ot + 1])
                        sk1 = rot.tile([128, NCH], LDT, name="sk1")
                        nc.vector.tensor_add(sk1[:], sk0[:], sh[:, ot, :])
                        nc.sync.dma_start(sk_halfD[qc, ot * 128:(ot + 1) * 128, :], sk1[:])
                    nc.gpsimd.collective_compute(
                        "AllGather", ALU.bypass, replica_groups=pairs,
                        ins=[sk_halfD[qc].opt()], outs=[sk_fullD[qc].opt()])

                carry = head1(0)
                for qc in range(1, QC):
                    nxt = head1(qc)
                    tail1(qc - 1, *carry)
                    carry = nxt
                tail1(QC - 1, *carry)

        # =========================== PHASE 2 ===========================
        with ExitStack() as ph2:
            wpool2 = ph2.enter_context(tc.tile_pool(name="w_ph2", bufs=1))
            res2 = ph2.enter_context(tc.tile_pool(name="res2", bufs=1))
            Wf = [wpool2.tile([128, C], LDT, name=f"Wf_{i}") for i in range(CT)]
            for i in range(CT):
                nc.sync.dma_start(Wf[i][:], WT["f"][i * 128:(i + 1) * 128, :])
            gkres = [[res2.tile([128, MW], LDT, name=f"gkres{i}_{j}") for j in range(MCH)]
                     for i in range(CT)]
            hvres = res2.tile([128, MT, C], VDT, name="hvres")
            hv2res = res2.tile([128, MT, C], VDT, name="hv2res")

            # ---- phase 2a: Gk + HvT + Hv2T build ----
            with ExitStack() as ph2a:
                wb2 = ph2a.enter_context(tc.tile_pool(name="w_build2", bufs=1))
                rot2a = ph2a.enter_context(tc.tile_pool(name="rot2a", bufs=2))
                Wg = [wb2.tile([128, C], LDT, name=f"Wg_{i}") for i in range(CT)]
                Wh = [wb2.tile([128, C], LDT, name=f"Wh_{i}") for i in range(CT)]
                for i in range(CT):
                    nc.sync.dma_start(Wg[i][:], WT["g"][i * 128:(i + 1) * 128, :])
                    nc.sync.dma_start(Wh[i][:], WT["h"][i * 128:(i + 1) * 128, :])
                for mch in range(MCH):
                    strot = [rot2a.tile([128, MW], LDT, name=f"strot{it}") for it in range(CT)]
                    for it in range(CT):
                        nc.gpsimd.dma_start(
                            strot[it][:], styleD[it * 128:(it + 1) * 128, mch * MW:(mch + 1) * MW])
                    for ms in range(MW // 128):
                        mt = mch * (MW // 128) + ms
                        psh = psum.tile([128, C], dt.float32, name="ps_hv", tag="big4", bufs=3)
                        for it in range(CT):
                            nc.tensor.matmul(psh[:], strot[it][:, ms * 128:(ms + 1) * 128],
                                             Wh[it][:], start=(it == 0), stop=(it == CT - 1))
                        nc.vector.tensor_add(hvres[:, mt, :], psh[:], brow["h"][:])
                        nc.gpsimd.tensor_mul(hv2res[:, mt, :], hvres[:, mt, :], hvres[:, mt, :])
                    hh = mch // (MCH // 2)
                    for s in range(MW // NCH):
                        qc2 = (mch % (MCH // 2)) * (MW // NCH) + s
                        skrot = [rot2a.tile([128, NCH], LDT, name=f"skrot{it}") for it in range(CT)]
                        for it in range(CT):
                            nc.sync.dma_start(
                                skrot[it][:], sk_fullD[qc2, hh, it * 128:(it + 1) * 128, :])
                        for ot in range(CT):
                            ps = psum.tile([128, NCH], dt.float32, name="ps_gk", tag="big4", bufs=3)
                            for it in range(CT):
                                nc.tensor.matmul(ps[:], Wg[it][:, ot * 128:(ot + 1) * 128],
                                                 skrot[it][:], start=(it == 0), stop=(it == CT - 1))
                            gv = gkres[ot][mch][:, s * NCH:(s + 1) * NCH]
                            if ot % 2 == 0:
                                nc.scalar.activation(gv, ps[:], AF.Identity,
                                                     bias=bcol["g"][:, ot:ot + 1])
                            else:
                                nc.vector.tensor_scalar_add(gv, ps[:],
                                                            bcol["g"][:, ot:ot + 1])

            # ---- phase 2b: attention 2 ----
            with ExitStack() as ph2b:
                rot2 = ph2b.enter_context(tc.tile_pool(name="rot2b", bufs=2))
                spool2 = ph2b.enter_context(tc.tile_pool(name="small2", bufs=2))

                def head2(qc):
                    cn = rot2.tile([128, CT, NCH], LDT, name="cn", bufs=2)
                    for it in range(CT):
                        crot2 = rot2.tile([128, NCH], dt.float32, name="crot2")
                        nc.gpsimd.dma_start(
                            crot2[:], cont_hD[it * 128:(it + 1) * 128, qc * NCH:(qc + 1) * NCH])
                        nc.vector.tensor_scalar(
                            out=cn[:, it, :], in0=crot2[:],
                            scalar1=mean_t[:, it:it + 1], scalar2=rstd_t[:, it:it + 1],
                            op0=ALU.subtract, op1=ALU.mult)
                    fq = rot2.tile([128, CT, NCH], LDT, name="fq", bufs=2)
                    for ot in range(CT):
                        psq = psum.tile([128, NCH], dt.float32, name="ps_fq", tag="big4", bufs=3)
                        for it in range(CT):
                            nc.tensor.matmul(psq[:], Wf[it][:, ot * 128:(ot + 1) * 128],
                                             cn[:, it, :], start=(it == 0), stop=(it == CT - 1))
                        nc.vector.tensor_scalar_add(fq[:, ot, :], psq[:], bcol["f"][:, ot:ot + 1])
                    sm = _sm_head(nc, psum, rot2, spool2, fq,
                                  lambda ct, j: gkres[ct][j][:], ident_h, name="2")
                    return cn, sm

                def tail2(qc, cn, sm):
                    LT = _sm_tailT(nc, psum, rot2, sm, name="2")
                    # mean then second (sequential to stay within PSUM)
                    mean_sb = rot2.tile([128, CT, NCH], dt.float32, name="mean_sb", bufs=1)
                    for ct in range(CT):
                        ps_m = psum.tile([128, NCH], dt.float32, name="ps_m", tag="big4", bufs=3)
                        for mt in range(MT):
                            nc.tensor.matmul(ps_m[:], hvres[:, mt, ct * 128:(ct + 1) * 128],
                                             LT[:, mt, :], start=(mt == 0), stop=(mt == MT - 1))
                        nc.vector.tensor_copy(mean_sb[:, ct, :], ps_m[:])
                    out_c = rot2.tile([128, CT, NCH], dt.float32, name="out_c", bufs=2)
                    for ct in range(CT):
                        ps_s = psum.tile([128, NCH], dt.float32, name="ps_s", tag="big4", bufs=3)
                        for mt in range(MT):
                            nc.tensor.matmul(ps_s[:], hv2res[:, mt, ct * 128:(ct + 1) * 128],
                                             LT[:, mt, :], start=(mt == 0), stop=(mt == MT - 1))
                        m2 = rot2.tile([128, NCH], dt.float32, name="m2")
                        nc.vector.tensor_mul(m2[:], mean_sb[:, ct, :], mean_sb[:, ct, :])
                        nc.vector.tensor_sub(m2[:], ps_s[:], m2[:])
                        nc.vector.tensor_scalar_max(m2[:], m2[:], 0.0)
                        nc.scalar.activation(m2[:], m2[:], AF.Sqrt, bias=zero_b[:])
                        nc.vector.tensor_mul(m2[:], m2[:], cn[:, ct, :])
                        nc.vector.tensor_add(out_c[:, ct, :], m2[:], mean_sb[:, ct, :])
                        nc.sync.dma_start(
                            outD[ct * 128:(ct + 1) * 128, qc * NCH:(qc + 1) * NCH],
                            out_c[:, ct, :])

                carry = head2(0)
                for qc in range(1, QC):
                    nxt = head2(qc)
                    tail2(qc - 1, *carry)
                    carry = nxt
                tail2(QC - 1, *carry)

    nc.compile()
    return nc


def _sm_head(nc, psum, rot, spool, q, krhs, ident_h, name):
    """Per n-chunk of NCH queries: logits [n, m] -> flash softmax exp tiles +
    per-(n,j) normalization diag matrices. q: [128, CT, NCH] LDT (lhsT slices);
    krhs(ct, j) -> [128, MW] LDT rhs AP for m-chunk j."""
    Texp = rot.tile([128, NQ, NS], VDT, name=f"Texp{name}", bufs=2)
    negmx = spool.tile([128, NQ, MCH], dt.float32, name=f"negmx{name}", bufs=2)
    ssum = spool.tile([128, NQ, MCH], dt.float32, name=f"ssum{name}", bufs=2)
    for nq in range(NQ):
        for j in range(MCH):
            psL = psum.tile([128, MW], dt.float32, name=f"psL{name}", tag="L", bufs=3)
            for ct in range(CT):
                nc.tensor.matmul(psL[:], q[:, ct, nq * 128:(nq + 1) * 128],
                                 krhs(ct, j), start=(ct == 0), stop=(ct == CT - 1))
            nc.vector.reduce_max(negmx[:, nq, j:j + 1], psL[:], axis=AX.X, negate=True)
            nc.scalar.activation(Texp[:, nq, j * MW:(j + 1) * MW], psL[:], AF.Exp,
                                 bias=negmx[:, nq, j:j + 1], accum_out=ssum[:, nq, j:j + 1])
    # factors: f_j = exp(mx_j - M); total = sum_j ssum_j * f_j; fr = f / total
    negM = spool.tile([128, NQ], dt.float32, name=f"negM{name}", bufs=2)
    f = spool.tile([128, NQ, MCH], dt.float32, name=f"fct{name}", bufs=2)
    sf = spool.tile([128, NQ, MCH], dt.float32, name=f"sf{name}", bufs=2)
    T = spool.tile([128, NQ], dt.float32, name=f"T{name}", bufs=2)
    r = spool.tile([128, NQ], dt.float32, name=f"r{name}", bufs=2)
    fr = spool.tile([128, NQ, MCH], dt.float32, name=f"fr{name}", bufs=2)
    diag = spool.tile([128, NQ, MCH, 128], VDT, name=f"diag{name}", bufs=2)
    for nq in range(NQ):
        nc.vector.tensor_reduce(negM[:, nq:nq + 1], negmx[:, nq, :], axis=AX.X, op=ALU.min)
        nc.scalar.activation(f[:, nq, :], negmx[:, nq, :], AF.Exp,
                             scale=-1.0, bias=negM[:, nq:nq + 1])
        nc.vector.tensor_mul(sf[:, nq, :], ssum[:, nq, :], f[:, nq, :])
        nc.vector.reduce_sum(T[:, nq:nq + 1], sf[:, nq, :], axis=AX.X)
        nc.vector.reciprocal(r[:, nq:nq + 1], T[:, nq:nq + 1])
        nc.vector.tensor_scalar_mul(fr[:, nq, :], f[:, nq, :], r[:, nq:nq + 1])
        for j in range(MCH):
            nc.vector.tensor_scalar_mul(diag[:, nq, j, :], ident_h[:], fr[:, nq, j:j + 1])
    return Texp, diag


def _sm_tailT(nc, psum, rot, sm, name):
    """Transpose with folded normalization: LT[m, n] = Texp[n, m] * fr[n]."""
    Texp, diag = sm
    LT = rot.tile([128, MT, NCH], VDT, name=f"LT{name}", bufs=1)
    for nq in range(NQ):
        for j in range(MCH):
            psT = psum.tile([128, 4, 128], dt.float32, name=f"psT{name}", tag="T", bufs=2)
            for sm_i in range(4):
                mt = j * 4 + sm_i
                nc.tensor.matmul(psT[:, sm_i, :], Texp[:, nq, mt * 128:(mt + 1) * 128],
                                 diag[:, nq, j, :], start=True, stop=True)
            if j % 2 == 0:
                nc.vector.tensor_copy(LT[:, j * 4:(j + 1) * 4, nq * 128:(nq + 1) * 128], psT[:])
            else:
                nc.scalar.copy(LT[:, j * 4:(j + 1) * 4, nq * 128:(nq + 1) * 128], psT[:])
    return LT


# ======================= host-side wrapper =======================

def prep_in_maps(inputs, n_cores=8):
    content = np.asarray(inputs['content'], np.float32)
    style = np.asarray(inputs['style'], np.float32)
    aes = np.asarray(inputs['aesthetic_feats'], np.float32)
    B = content.shape[0]
    content = content.reshape(B, C, -1)
    style = style.reshape(B, C, -1)
    aes = aes.reshape(B, C, -1)
    wmap = {'f1': 'Wf1', 'f2': 'Wf2', 'f3': 'Wf3', 'frs': 'Wfrs', 'f': 'Wf', 'g': 'Wg', 'h': 'Wh'}
    bmap = {'f1': 'bf1', 'f2': 'bf2', 'f3': 'bf3', 'frs': 'bfrs', 'f': 'bf', 'g': 'bg', 'h': 'bh'}
    const = {'ident': np.eye(128, dtype=np.float32)}
    for k, wn in wmap.items():
        const[f'WT{k}'] = np.ascontiguousarray(np.asarray(inputs[wn], np.float32).T.astype(np.float16))
    for k in ['f1', 'f2', 'frs', 'f', 'g']:
        const[f'bcol{k}'] = np.ascontiguousarray(
            np.asarray(inputs[bmap[k]], np.float32).reshape(CT, 128).T)
    for k in ['f3', 'h']:
        const[f'brow{k}'] = np.asarray(inputs[bmap[k]], np.float32).reshape(1, C)
    in_maps = []
    for c in range(n_cores):
        b, h = c // 2, c % 2
        m = dict(const)
        m['style'] = np.ascontiguousarray(style[b].astype(np.float16))
        m['style_h'] = np.ascontiguousarray(style[b][:, h * NH:(h + 1) * NH].astype(np.float16))
        m['aes'] = np.ascontiguousarray(aes[b].astype(np.float16))
        m['cont_h'] = np.ascontiguousarray(content[b][:, h * NH:(h + 1) * NH])
        in_maps.append(m)
    return in_maps


def assemble_out(results, n_cores=8, H=64, W=64):
    B = n_cores // 2
    out = np.empty((B, C, NS), np.float32)
    for c in range(n_cores):
        b, h = c // 2, c % 2
        out[b][:, h * NH:(h + 1) * NH] = results[c]['out']
    return out.reshape(B, C, H, W)


# ======================= harness entry point =======================

_CACHE = {}


def kernel(**inputs):
    """Full-input AesSA kernel on 8 NeuronCores (2 cores per sample,
    query-axis sharding). Returns [4, 512, 64, 64] float32."""
    from concourse.bass_utils import run_bass_kernel_spmd
    if 'nc' not in _CACHE:
        _CACHE['nc'] = build(n_cores=8)
    nc = _CACHE['nc']
    in_maps = prep_in_maps(inputs, n_cores=8)
    res = run_bass_kernel_spmd(nc, in_maps, list(range(8)))
    return assemble_out(res.results, n_cores=8, H=64, W=64)


# revision 12
# speedup vs baseline: 1.0418x; 1.0418x over previous
"""AesSA Trainium kernel v2: 2 cores per sample, query-axis sharded attention.

Key differences from v1:
- k/vT (phase 1) and gk/hvT/hv2T (phase 2) are SBUF-resident (no DRAM round trips).
- Logits in [n, m] layout (queries in partitions): rowmax/exp/rowsum are
  per-partition ops (DVE reduce_max negate, Act Exp with bias + accum_out).
- Flash-style per-(n-chunk, m-chunk) local max; normalization and the
  exp(local-max) rescale folded into the transpose step as a diagonal-matrix
  matmul (LT = Texp^T @ diag(f_j/rowsum)).
- Value-path matmuls (transpose, AV, mean, second) in fp16.
- Content stats computed from this core's half + pairwise AllGather.
"""
import sys
sys.path.insert(0, '/opt/trn_rl_repo')
import numpy as np
import concourse.bacc as bacc
import concourse.mybir as mybir
import concourse.tile as tile
from contextlib import ExitStack

dt = mybir.dt
AF = mybir.ActivationFunctionType
AX = mybir.AxisListType
ALU = mybir.AluOpType

C = 512
CT = 4
EPS = 1e-5
LDT = dt.float16    # logits-path matmul operand dtype
VDT = dt.float16    # value-path matmul operand dtype

NS = 4096
NH = NS // 2
MT = NS // 128      # 32 m-tiles
MW = 512            # m-chunk width (f32r needs >=256 free for full rate)
MCH = NS // MW      # 8 m-chunks
NCH = 256           # n-chunk width
NQ = NCH // 128     # 2 n-tiles per n-chunk
QC = NH // NCH      # 8 n-chunks per core


def build(n_cores=8, pairs=None):
    if pairs is None:
        pairs = [[2 * i, 2 * i + 1] for i in range(n_cores // 2)]

    nc = bacc.Bacc("TRN2", target_bir_lowering=False, debug=False, num_devices=n_cores)

    styleD = nc.dram_tensor("style", [C, NS], LDT, kind="ExternalInput")
    style_hD = nc.dram_tensor("style_h", [C, NH], LDT, kind="ExternalInput")
    aesD = nc.dram_tensor("aes", [C, NS], LDT, kind="ExternalInput")
    cont_hD = nc.dram_tensor("cont_h", [C, NH], dt.float32, kind="ExternalInput")
    WT = {}
    for w in ["f1", "f2", "f3", "frs", "f", "g", "h"]:
        WT[w] = nc.dram_tensor(f"WT{w}", [C, C], LDT, kind="ExternalInput")
    BCOL = {}
    for w in ["f1", "f2", "frs", "f", "g"]:
        BCOL[w] = nc.dram_tensor(f"bcol{w}", [128, CT], dt.float32, kind="ExternalInput")
    BROW = {}
    for w in ["f3", "h"]:
        BROW[w] = nc.dram_tensor(f"brow{w}", [1, C], dt.float32, kind="ExternalInput")
    identD = nc.dram_tensor("ident", [128, 128], dt.float32, kind="ExternalInput")
    outD = nc.dram_tensor("out", [C, NH], dt.float32, kind="ExternalOutput")

    with tile.TileContext(nc, num_cores=n_cores) as tc, ExitStack() as octx:
        dram = octx.enter_context(tc.tile_pool(name="dram", bufs=1, space="DRAM"))
        sk_halfD = dram.tile([QC, C, NCH], LDT, name="sk_halfD")
        sk_fullD = dram.tile([QC, 2, C, NCH], LDT, name="sk_fullD")
        statPD = dram.tile([128, 2 * CT], dt.float32, name="statPD")
        statFD = dram.tile([2, 128, 2 * CT], dt.float32, name="statFD")

        cpool = octx.enter_context(tc.tile_pool(name="const", bufs=1))
        psum = octx.enter_context(tc.tile_pool(name="psum", bufs=1, space="PSUM"))

        # ---- constants ----
        ident = cpool.tile([128, 128], dt.float32, name="ident")
        nc.sync.dma_start(ident[:], identD[:])
        ident_h = cpool.tile([128, 128], VDT, name="ident_h")
        nc.vector.tensor_copy(ident_h[:], ident[:])
        bcol = {}
        for w in BCOL:
            bcol[w] = cpool.tile([128, CT], dt.float32, name=f"bcol{w}")
            nc.sync.dma_start(bcol[w][:], BCOL[w][:])
        brow = {}
        for w in BROW:
            r = cpool.tile([1, C], dt.float32, name=f"brow{w}_r")
            nc.sync.dma_start(r[:], BROW[w][:])
            brow[w] = cpool.tile([128, C], dt.float32, name=f"brow{w}")
            nc.gpsimd.partition_broadcast(brow[w][:], r[:])
        zero_b = cpool.tile([128, 1], dt.float32, name="zero_b")
        nc.gpsimd.memset(zero_b[:], 0.0)
        eps_b = cpool.tile([128, 1], dt.float32, name="eps_b")
        nc.gpsimd.memset(eps_b[:], EPS)
        mean_t = cpool.tile([128, CT], dt.float32, name="mean_t")
        rstd_t = cpool.tile([128, CT], dt.float32, name="rstd_t")

        # =========================== PHASE 1 ===========================
        with ExitStack() as ph1:
            wpool = ph1.enter_context(tc.tile_pool(name="w_ph1", bufs=1))
            res1 = ph1.enter_context(tc.tile_pool(name="res1", bufs=1))
            Wf1 = [wpool.tile([128, C], LDT, name=f"Wf1_{i}") for i in range(CT)]
            Wfrs = [wpool.tile([128, C], LDT, name=f"Wfrs_{i}") for i in range(CT)]
            for i in range(CT):
                nc.sync.dma_start(Wf1[i][:], WT["f1"][i * 128:(i + 1) * 128, :])
                nc.sync.dma_start(Wfrs[i][:], WT["frs"][i * 128:(i + 1) * 128, :])
            kres = [[res1.tile([128, MW], LDT, name=f"kres{i}_{j}") for j in range(MCH)]
                    for i in range(CT)]
            vres = res1.tile([128, MT, C], VDT, name="vres")

            # ---- phase 1a: k + vT build, content-half stats ----
            with ExitStack() as ph1a:
                wb = ph1a.enter_context(tc.tile_pool(name="w_build", bufs=1))
                rota = ph1a.enter_context(tc.tile_pool(name="rot1a", bufs=2))
                Wf2 = [wb.tile([128, C], LDT, name=f"Wf2_{i}") for i in range(CT)]
                Wf3 = [wb.tile([128, C], LDT, name=f"Wf3_{i}") for i in range(CT)]
                for i in range(CT):
                    nc.sync.dma_start(Wf2[i][:], WT["f2"][i * 128:(i + 1) * 128, :])
                    nc.sync.dma_start(Wf3[i][:], WT["f3"][i * 128:(i + 1) * 128, :])
                for mch in range(MCH):
                    arot = [rota.tile([128, MW], LDT, name=f"arot{it}") for it in range(CT)]
                    for it in range(CT):
                        nc.gpsimd.dma_start(
                            arot[it][:], aesD[it * 128:(it + 1) * 128, mch * MW:(mch + 1) * MW])
                    for ot in range(CT):
                        ps = psum.tile([128, MW], dt.float32, name="ps_k", tag="big4", bufs=3)
                        for it in range(CT):
                            nc.tensor.matmul(ps[:], Wf2[it][:, ot * 128:(ot + 1) * 128],
                                             arot[it][:], start=(it == 0), stop=(it == CT - 1))
                        nc.scalar.activation(kres[ot][mch][:], ps[:],
                                             AF.Identity, bias=bcol["f2"][:, ot:ot + 1])
                    for ms in range(MW // 128):
                        mt = mch * (MW // 128) + ms
                        psv = psum.tile([128, C], dt.float32, name="ps_v", tag="big4", bufs=3)
                        for it in range(CT):
                            nc.tensor.matmul(psv[:], arot[it][:, ms * 128:(ms + 1) * 128],
                                             Wf3[it][:], start=(it == 0), stop=(it == CT - 1))
                        nc.vector.tensor_add(vres[:, mt, :], psv[:], brow["f3"][:])
                # content-half stats + pairwise exchange
                NSC = NH // MW  # 4 chunks of 512 for stats
                sumst = rota.tile([128, CT, NSC], dt.float32, name="sumst", bufs=1)
                sqst = rota.tile([128, CT, NSC], dt.float32, name="sqst", bufs=1)
                for sc in range(NSC):
                    for ct in range(CT):
                        crot = rota.tile([128, MW], dt.float32, name="crot", bufs=3)
                        nc.sync.dma_start(
                            crot[:], cont_hD[ct * 128:(ct + 1) * 128, sc * MW:(sc + 1) * MW])
                        nc.vector.reduce_sum(sumst[:, ct, sc:sc + 1], crot[:], axis=AX.X)
                        sq_scr = rota.tile([128, MW], dt.float32, name="sq_scr", bufs=3)
                        nc.scalar.activation(sq_scr[:], crot[:], AF.Square, bias=zero_b[:],
                                             accum_out=sqst[:, ct, sc:sc + 1])
                statP = rota.tile([128, 2 * CT], dt.float32, name="statP", bufs=1)
                for ct in range(CT):
                    nc.vector.reduce_sum(statP[:, ct:ct + 1], sumst[:, ct, :], axis=AX.X)
                    nc.vector.reduce_sum(statP[:, CT + ct:CT + ct + 1], sqst[:, ct, :], axis=AX.X)
                nc.sync.dma_start(statPD[:], statP[:])
                nc.gpsimd.collective_compute(
                    "AllGather", ALU.bypass, replica_groups=pairs,
                    ins=[statPD[:].opt()], outs=[statFD[:].opt()])
                statF = rota.tile([128, 2, 2 * CT], dt.float32, name="statF", bufs=1)
                for hh in range(2):
                    nc.sync.dma_start(statF[:, hh, :], statFD[hh])
                tot = rota.tile([128, 2 * CT], dt.float32, name="tot", bufs=1)
                nc.vector.tensor_add(tot[:], statF[:, 0, :], statF[:, 1, :])
                nc.vector.tensor_scalar_mul(mean_t[:], tot[:, 0:CT], 1.0 / NS)
                ex2 = rota.tile([128, CT], dt.float32, name="ex2", bufs=1)
                nc.vector.tensor_scalar_mul(ex2[:], tot[:, CT:2 * CT], 1.0 / NS)
                msq = rota.tile([128, CT], dt.float32, name="msq_t", bufs=1)
                nc.vector.tensor_mul(msq[:], mean_t[:], mean_t[:])
                var_t = rota.tile([128, CT], dt.float32, name="var_t", bufs=1)
                nc.vector.tensor_sub(var_t[:], ex2[:], msq[:])
                sd_t = rota.tile([128, CT], dt.float32, name="sd_t", bufs=1)
                nc.scalar.activation(sd_t[:], var_t[:], AF.Sqrt, bias=eps_b[:])
                nc.vector.reciprocal(rstd_t[:], sd_t[:])

            # ---- phase 1b: attention 1, [n, m] layout ----
            with ExitStack() as ph1b:
                rot = ph1b.enter_context(tc.tile_pool(name="rot1b", bufs=2))
                spool = ph1b.enter_context(tc.tile_pool(name="small1", bufs=2))

                def head1(qc):
                    sh = rot.tile([128, CT, NCH], LDT, name="sh", bufs=2)
                    for it in range(CT):
                        nc.gpsimd.dma_start(
                            sh[:, it, :], style_hD[it * 128:(it + 1) * 128, qc * NCH:(qc + 1) * NCH])
                    q = rot.tile([128, CT, NCH], LDT, name="q", bufs=2)
                    for ot in range(CT):
                        psq = psum.tile([128, NCH], dt.float32, name="ps_q", tag="big4", bufs=3)
                        for it in range(CT):
                            nc.tensor.matmul(psq[:], Wf1[it][:, ot * 128:(ot + 1) * 128],
                                             sh[:, it, :], start=(it == 0), stop=(it == CT - 1))
                        nc.vector.tensor_scalar_add(q[:, ot, :], psq[:], bcol["f1"][:, ot:ot + 1])
                    sm = _sm_head(nc, psum, rot, spool, q,
                                  lambda ct, j: kres[ct][j][:], ident_h, name="1")
                    return sh, sm

                def tail1(qc, sh, sm):
                    LT = _sm_tailT(nc, psum, rot, sm, name="1")
                    # AV: xn[c, n] = sum_m vT[m, c] * LT[m, n]  (pre-normalized)
                    xn = rot.tile([128, CT, NCH], LDT, name="xn", bufs=1)
                    for ct in range(CT):
                        ps_av = psum.tile([128, NCH], dt.float32, name="ps_av", tag="big4", bufs=3)
                        for mt in range(MT):
                            nc.tensor.matmul(ps_av[:], vres[:, mt, ct * 128:(ct + 1) * 128],
                                             LT[:, mt, :], start=(mt == 0), stop=(mt == MT - 1))
                        nc.vector.tensor_copy(xn[:, ct, :], ps_av[:])
                    # Wfrs conv + residual -> sk chunk -> collective
                    for ot in range(CT):
                        ps = psum.tile([128, NCH], dt.float32, name="ps_sk", tag="big4", bufs=3)
                        for it in range(CT):
                            nc.tensor.matmul(ps[:], Wfrs[it][:, ot * 128:(ot + 1) * 128],
                                             xn[:, it, :], start=(it == 0), stop=(it == CT - 1))
                        sk0 = rot.tile([128, NCH], dt.float32, name="sk0")
                        nc.vector.tensor_scalar_add(sk0[:], ps[:], bcol["frs"][:, ot:ot + 1])
                        sk1 = rot.tile([128, NCH], LDT, name="sk1")
                        nc.vector.tensor_add(sk1[:], sk0[:], sh[:, ot, :])
                        nc.sync.dma_start(sk_halfD[qc, ot * 128:(ot + 1) * 128, :], sk1[:])
                    nc.gpsimd.collective_compute(
                        "AllGather", ALU.bypass, replica_groups=pairs,
                        ins=[sk_halfD[qc].opt()], outs=[sk_fullD[qc].opt()])

                carry = head1(0)
                for qc in range(1, QC):
                    nxt = head1(qc)
                    tail1(qc - 1, *carry)
                    carry = nxt
                tail1(QC - 1, *carry)

        # =========================== PHASE 2 ===========================
        with ExitStack() as ph2:
            wpool2 = ph2.enter_context(tc.tile_pool(name="w_ph2", bufs=1))
            res2 = ph2.enter_context(tc.tile_pool(name="res2", bufs=1))
            Wf = [wpool2.tile([128, C], LDT, name=f"Wf_{i}") for i in range(CT)]
            for i in range(CT):
                nc.sync.dma_start(Wf[i][:], WT["f"][i * 128:(i + 1) * 128, :])
            gkres = [[res2.tile([128, MW], LDT, name=f"gkres{i}_{j}") for j in range(MCH)]
                     for i in range(CT)]
            hvres = res2.tile([128, MT, C], VDT, name="hvres")
            hv2res = res2.tile([128, MT, C], VDT, name="hv2res")

            # ---- phase 2a: Gk + HvT + Hv2T build ----
            with ExitStack() as ph2a:
                wb2 = ph2a.enter_context(tc.tile_pool(name="w_build2", bufs=1))
                rot2a = ph2a.enter_context(tc.tile_pool(name="rot2a", bufs=2))
                Wg = [wb2.tile([128, C], LDT, name=f"Wg_{i}") for i in range(CT)]
                Wh = [wb2.tile([128, C], LDT, name=f"Wh_{i}") for i in range(CT)]
                for i in range(CT):
                    nc.sync.dma_start(Wg[i][:], WT["g"][i * 128:(i + 1) * 128, :])
                    nc.sync.dma_start(Wh[i][:], WT["h"][i * 128:(i + 1) * 128, :])
                for mch in range(MCH):
                    strot = [rot2a.tile([128, MW], LDT, name=f"strot{it}") for it in range(CT)]
                    for it in range(CT):
                        nc.gpsimd.dma_start(
                            strot[it][:], styleD[it * 128:(it + 1) * 128, mch * MW:(mch + 1) * MW])
                    for ms in range(MW // 128):
                        mt = mch * (MW // 128) + ms
                        psh = psum.tile([128, C], dt.float32, name="ps_hv", tag="big4", bufs=3)
                        for it in range(CT):
                            nc.tensor.matmul(psh[:], strot[it][:, ms * 128:(ms + 1) * 128],
                                             Wh[it][:], start=(it == 0), stop=(it == CT - 1))
                        nc.vector.tensor_add(hvres[:, mt, :], psh[:], brow["h"][:])
                        nc.gpsimd.tensor_mul(hv2res[:, mt, :], hvres[:, mt, :], hvres[:, mt, :])
                    hh = mch // (MCH // 2)
                    for s in range(MW // NCH):
                        qc2 = (mch % (MCH // 2)) * (MW // NCH) + s
                        skrot = [rot2a.tile([128, NCH], LDT, name=f"skrot{it}") for it in range(CT)]
                        for it in range(CT):
                            nc.sync.dma_start(
                                skrot[it][:], sk_fullD[qc2, hh, it * 128:(it + 1) * 128, :])
                        for ot in range(CT):
                            ps = psum.tile([128, NCH], dt.float32, name="ps_gk", tag="big4", bufs=3)
                            for it in range(CT):
                                nc.tensor.matmul(ps[:], Wg[it][:, ot * 128:(ot + 1) * 128],
                                                 skrot[it][:], start=(it == 0), stop=(it == CT - 1))
                            nc.scalar.activation(gkres[ot][mch][:, s * NCH:(s + 1) * NCH],
                                                 ps[:], AF.Identity,
                                                 bias=bcol["g"][:, ot:ot + 1])

            # ---- phase 2b: attention 2 ----
            with ExitStack() as ph2b:
                rot2 = ph2b.enter_context(tc.tile_pool(name="rot2b", bufs=2))
                spool2 = ph2b.enter_context(tc.tile_pool(name="small2", bufs=2))

                def head2(qc):
                    cn = rot2.tile([128, CT, NCH], LDT, name="cn", bufs=2)
                    for it in range(CT):
                        crot2 = rot2.tile([128, NCH], dt.float32, name="crot2")
                        nc.gpsimd.dma_start(
                            crot2[:], cont_hD[it * 128:(it + 1) * 128, qc * NCH:(qc + 1) * NCH])
                        nc.vector.tensor_scalar(
                            out=cn[:, it, :], in0=crot2[:],
                            scalar1=mean_t[:, it:it + 1], scalar2=rstd_t[:, it:it + 1],
                            op0=ALU.subtract, op1=ALU.mult)
                    fq = rot2.tile([128, CT, NCH], LDT, name="fq", bufs=2)
                    for ot in range(CT):
                        psq = psum.tile([128, NCH], dt.float32, name="ps_fq", tag="big4", bufs=3)
                        for it in range(CT):
                            nc.tensor.matmul(psq[:], Wf[it][:, ot * 128:(ot + 1) * 128],
                                             cn[:, it, :], start=(it == 0), stop=(it == CT - 1))
                        nc.vector.tensor_scalar_add(fq[:, ot, :], psq[:], bcol["f"][:, ot:ot + 1])
                    sm = _sm_head(nc, psum, rot2, spool2, fq,
                                  lambda ct, j: gkres[ct][j][:], ident_h, name="2")
                    return cn, sm

                def tail2(qc, cn, sm):
                    LT = _sm_tailT(nc, psum, rot2, sm, name="2")
                    # mean then second (sequential to stay within PSUM)
                    mean_sb = rot2.tile([128, CT, NCH], dt.float32, name="mean_sb", bufs=1)
                    for ct in range(CT):
                        ps_m = psum.tile([128, NCH], dt.float32, name="ps_m", tag="big4", bufs=3)
                        for mt in range(MT):
                            nc.tensor.matmul(ps_m[:], hvres[:, mt, ct * 128:(ct + 1) * 128],
                                             LT[:, mt, :], start=(mt == 0), stop=(mt == MT - 1))
                        nc.vector.tensor_copy(mean_sb[:, ct, :], ps_m[:])
                    out_c = rot2.tile([128, CT, NCH], dt.float32, name="out_c", bufs=2)
                    for ct in range(CT):
                        ps_s = psum.tile([128, NCH], dt.float32, name="ps_s", tag="big4", bufs=3)
                        for mt in range(MT):
                            nc.tensor.matmul(ps_s[:], hv2res[:, mt, ct * 128:(ct + 1) * 128],
                                             LT[:, mt, :], start=(mt == 0), stop=(mt == MT - 1))
                        m2 = rot2.tile([128, NCH], dt.float32, name="m2")
                        nc.vector.tensor_mul(m2[:], mean_sb[:, ct, :], mean_sb[:, ct, :])
                        nc.vector.tensor_sub(m2[:], ps_s[:], m2[:])
                        nc.vector.tensor_scalar_max(m2[:], m2[:], 0.0)
                        nc.scalar.activation(m2[:], m2[:], AF.Sqrt, bias=zero_b[:])
                        nc.vector.tensor_mul(m2[:], m2[:], cn[:, ct, :])
                        nc.vector.tensor_add(out_c[:, ct, :], m2[:], mean_sb[:, ct, :])
                        nc.sync.dma_start(
                            outD[ct * 128:(ct + 1) * 128, qc * NCH:(qc + 1) * NCH],
                            out_c[:, ct, :])

                carry = head2(0)
                for qc in range(1, QC):
                    nxt = head2(qc)
                    tail2(qc - 1, *carry)
                    carry = nxt
                tail2(QC - 1, *carry)

    nc.compile()
    return nc


def _sm_head(nc, psum, rot, spool, q, krhs, ident_h, name):
    """Per n-chunk of NCH queries: logits [n, m] -> flash softmax exp tiles +
    per-(n,j) normalization diag matrices. q: [128, CT, NCH] LDT (lhsT slices);
    krhs(ct, j) -> [128, MW] LDT rhs AP for m-chunk j."""
    Texp = rot.tile([128, NQ, NS], VDT, name=f"Texp{name}", bufs=2)
    negmx = spool.tile([128, NQ, MCH], dt.float32, name=f"negmx{name}", bufs=2)
    ssum = spool.tile([128, NQ, MCH], dt.float32, name=f"ssum{name}", bufs=2)
    for nq in range(NQ):
        for j in range(MCH):
            psL = psum.tile([128, MW], dt.float32, name=f"psL{name}", tag="L", bufs=3)
            for ct in range(CT):
                nc.tensor.matmul(psL[:], q[:, ct, nq * 128:(nq + 1) * 128],
                                 krhs(ct, j), start=(ct == 0), stop=(ct == CT - 1))
            nc.vector.reduce_max(negmx[:, nq, j:j + 1], psL[:], axis=AX.X, negate=True)
            nc.scalar.activation(Texp[:, nq, j * MW:(j + 1) * MW], psL[:], AF.Exp,
                                 bias=negmx[:, nq, j:j + 1], accum_out=ssum[:, nq, j:j + 1])
    # factors: f_j = exp(mx_j - M); total = sum_j ssum_j * f_j; fr = f / total
    negM = spool.tile([128, NQ], dt.float32, name=f"negM{name}", bufs=2)
    f = spool.tile([128, NQ, MCH], dt.float32, name=f"fct{name}", bufs=2)
    sf = spool.tile([128, NQ, MCH], dt.float32, name=f"sf{name}", bufs=2)
    T = spool.tile([128, NQ], dt.float32, name=f"T{name}", bufs=2)
    r = spool.tile([128, NQ], dt.float32, name=f"r{name}", bufs=2)
    fr = spool.tile([128, NQ, MCH], dt.float32, name=f"fr{name}", bufs=2)
    diag = spool.tile([128, NQ, MCH, 128], VDT, name=f"diag{name}", bufs=2)
    for nq in range(NQ):
        nc.vector.tensor_reduce(negM[:, nq:nq + 1], negmx[:, nq, :], axis=AX.X, op=ALU.min)
        nc.scalar.activation(f[:, nq, :], negmx[:, nq, :], AF.Exp,
                             scale=-1.0, bias=negM[:, nq:nq + 1])
        nc.vector.tensor_mul(sf[:, nq, :], ssum[:, nq, :], f[:, nq, :])
        nc.vector.reduce_sum(T[:, nq:nq + 1], sf[:, nq, :], axis=AX.X)
        nc.vector.reciprocal(r[:, nq:nq + 1], T[:, nq:nq + 1])
        nc.vector.tensor_scalar_mul(fr[:, nq, :], f[:, nq, :], r[:, nq:nq + 1])
        for j in range(MCH):
            nc.vector.tensor_scalar_mul(diag[:, nq, j, :], ident_h[:], fr[:, nq, j:j + 1])
    return Texp, diag


def _sm_tailT(nc, psum, rot, sm, name):
    """Transpose with folded normalization: LT[m, n] = Texp[n, m] * fr[n]."""
    Texp, diag = sm
    LT = rot.tile([128, MT, NCH], VDT, name=f"LT{name}", bufs=1)
    for nq in range(NQ):
        for j in range(MCH):
            psT = psum.tile([128, 4, 128], dt.float32, name=f"psT{name}", tag="T", bufs=2)
            for sm_i in range(4):
                mt = j * 4 + sm_i
                nc.tensor.matmul(psT[:, sm_i, :], Texp[:, nq, mt * 128:(mt + 1) * 128],
                                 diag[:, nq, j, :], start=True, stop=True)
            if j % 2 == 0:
                nc.vector.tensor_copy(LT[:, j * 4:(j + 1) * 4, nq * 128:(nq + 1) * 128], psT[:])
            else:
                nc.scalar.copy(LT[:, j * 4:(j + 1) * 4, nq * 128:(nq + 1) * 128], psT[:])
    return LT


# ======================= host-side wrapper =======================

def prep_in_maps(inputs, n_cores=8):
    content = np.asarray(inputs['content'], np.float32)
    style = np.asarray(inputs['style'], np.float32)
    aes = np.asarray(inputs['aesthetic_feats'], np.float32)
    B = content.shape[0]
    content = content.reshape(B, C, -1)
    style = style.reshape(B, C, -1)
    aes = aes.reshape(B, C, -1)
    wmap = {'f1': 'Wf1', 'f2': 'Wf2', 'f3': 'Wf3', 'frs': 'Wfrs', 'f': 'Wf', 'g': 'Wg', 'h': 'Wh'}
    bmap = {'f1': 'bf1', 'f2': 'bf2', 'f3': 'bf3', 'frs': 'bfrs', 'f': 'bf', 'g': 'bg', 'h': 'bh'}
    const = {'ident': np.eye(128, dtype=np.float32)}
    for k, wn in wmap.items():
        const[f'WT{k}'] = np.ascontiguousarray(np.asarray(inputs[wn], np.float32).T.astype(np.float16))
    for k in ['f1', 'f2', 'frs', 'f', 'g']:
        const[f'bcol{k}'] = np.ascontiguousarray(
            np.asarray(inputs[bmap[k]], np.float32).reshape(CT, 128).T)
    for k in ['f3', 'h']:
        const[f'brow{k}'] = np.asarray(inputs[bmap[k]], np.float32).reshape(1, C)
    in_maps = []
    for c in range(n_cores):
        b, h = c // 2, c % 2
        m = dict(const)
        m['style'] = np.ascontiguousarray(style[b].astype(np.float16))
        m['style_h'] = np.ascontiguousarray(style[b][:, h * NH:(h + 1) * NH].astype(np.float16))
        m['aes'] = np.ascontiguousarray(aes[b].astype(np.float16))
        m['cont_h'] = np.ascontiguousarray(content[b][:, h * NH:(h + 1) * NH])
        in_maps.append(m)
    return in_maps


def assemble_out(results, n_cores=8, H=64, W=64):
    B = n_cores // 2
    out = np.empty((B, C, NS), np.float32)
    for c in range(n_cores):
        b, h = c // 2, c % 2
        out[b][:, h * NH:(h + 1) * NH] = results[c]['out']
    return out.reshape(B, C, H, W)


# ======================= harness entry point =======================

_CACHE = {}


def kernel(**inputs):
    """Full-input AesSA kernel on 8 NeuronCores (2 cores per sample,
    query-axis sharding). Returns [4, 512, 64, 64] float32."""
    from concourse.bass_utils import run_bass_kernel_spmd
    if 'nc' not in _CACHE:
        _CACHE['nc'] = build(n_cores=8)
    nc = _CACHE['nc']
    in_maps = prep_in_maps(inputs, n_cores=8)
    res = run_bass_kernel_spmd(nc, in_maps, list(range(8)))
    return assemble_out(res.results, n_cores=8, H=64, W=64)
